# revision 22
# baseline (speedup 1.0000x reference)
"""Trainium2 Bass kernel for nn_Attention_60576218743412.

LayerNorm -> QKV projection -> 2D axial RoPE -> full softmax attention ->
out-projection, for x[B=4, N=2048, D=768], 12 heads of 64.

Sharding: 8 cores = 4 batches x 2 head-groups (6 heads each).  Each core
computes LN + QKV for its 6 heads, attention, and a partial out-projection
(its 384 columns of w_out); the host sums the two partials per batch.

Restructure vs the v1 kernel (527 us):
- LayerNorm's mean subtraction is folded into the host-side projection
  weights (row-centering: W @ (x - mu) == (W - rowmean(W)) @ x), and the
  rsqrt-variance scale r[t] is folded into the RoPE cos/sin tables (q, k)
  and into the v PSUM->SBUF copy.  All projections run on RAW x, so QKV
  no longer waits on the LN stats.
- Softmax row-sums ride a 65th "ones" column next to each head's v.
- AV is token-major (out [qtok,65]): full-128-partition matmuls halve the
  PE streaming cost vs feature-major; the normalized output is transposed
  back with cheap bf16 PE transposes.
- exp runs on ACT at [128,1024] granularity (the hard floor, ~200us); the
  PE instruction stream is ordered so ACT never starves: scores for round
  k+1 are issued before AV of round k, and q-projection for the next token
  chunk is woven between attention rounds.
"""

import os

import numpy as np
import ml_dtypes

KDBG = bool(os.environ.get("KDBG"))

B, N, D = 4, 2048, 768
HEADS, DH = 12, 64
HG = 6                # heads per core
E = HG * DH           # 384: per-core qkv width
ROPE_BASE = 8192.0
LN_EPS = 1e-5
P = 128
DC = D // P           # 6 contraction chunks
ECH = E // P          # 3 e-chunks
TCH = 4               # token chunks for 512-wide matmuls
QW = N // TCH         # 512
KCH = N // P          # 16 key chunks
NPAIR = HG // 2       # head pairs per core

_GRAPH_CACHE = {}


def _build_graph():
    from contextlib import ExitStack

    import concourse.tile as tile
    from concourse import bacc, mybir

    f32 = mybir.dt.float32
    f32r = mybir.dt.float32r
    bf16 = mybir.dt.bfloat16
    AL = mybir.AluOpType
    AF = mybir.ActivationFunctionType

    nc = bacc.Bacc(None, target_bir_lowering=False)

    xT = nc.dram_tensor("xT", [TCH, P, DC, QW], f32r, kind="ExternalInput")
    wqT = nc.dram_tensor("wqT", [P, DC, E], f32r, kind="ExternalInput")
    wkT = nc.dram_tensor("wkT", [P, DC, E], f32r, kind="ExternalInput")
    wvT = nc.dram_tensor("wvT", [P, DC, E], f32r, kind="ExternalInput")
    woT = nc.dram_tensor("woT", [P, ECH, D], bf16, kind="ExternalInput")
    coordsT = nc.dram_tensor("coordsT", [2, N], f32, kind="ExternalInput")
    invf = nc.dram_tensor("invf", [P, 1], f32, kind="ExternalInput")
    permA = nc.dram_tensor("permA", [P, P], f32r, kind="ExternalInput")
    permB = nc.dram_tensor("permB", [P, P], f32r, kind="ExternalInput")
    identT = nc.dram_tensor("identT", [P, P], bf16, kind="ExternalInput")
    onesc = nc.dram_tensor("onesc", [P, 1], f32r, kind="ExternalInput")
    onesb = nc.dram_tensor("onesb", [P, 1], bf16, kind="ExternalInput")
    outT = nc.dram_tensor("out", [D, N], f32, kind="ExternalOutput")
    if KDBG:
        dbg_r = nc.dram_tensor("dbg_r", [P, KCH], f32, kind="ExternalOutput")
        dbg_cos = nc.dram_tensor("dbg_cos", [P, N], f32, kind="ExternalOutput")
        dbg_sin = nc.dram_tensor("dbg_sin", [P, N], f32, kind="ExternalOutput")
        dbg_kr = nc.dram_tensor("dbg_kr", [P, ECH, N], bf16, kind="ExternalOutput")
        dbg_v = nc.dram_tensor("dbg_v", [P, KCH, HG * 65], bf16, kind="ExternalOutput")
        dbg_q0 = nc.dram_tensor("dbg_q0", [P, ECH, QW], bf16, kind="ExternalOutput")
        dbg_at = nc.dram_tensor("dbg_at", [P, ECH, QW], bf16, kind="ExternalOutput")

    outT_r = outT.rearrange("(c p) t -> p c t", p=P)

    MAGIC = float(2.0 ** 23)
    TWO_PI = float(2 * np.pi)
    SCALE = float(DH ** -0.5)

    with tile.TileContext(nc) as tc, ExitStack() as octx:
        consts = octx.enter_context(tc.tile_pool(name="consts", bufs=1))
        persist = octx.enter_context(tc.tile_pool(name="persist", bufs=1))
        dram = octx.enter_context(tc.tile_pool(name="dram", bufs=1, space="DRAM"))

        # PSUM: sc 2 banks x2 + av 1 bank x2 + tpop 1 bank x2 = 8 banks
        scp = octx.enter_context(tc.tile_pool(name="sc_ps", bufs=2, space="PSUM"))
        avp = octx.enter_context(tc.tile_pool(name="av_ps", bufs=2, space="PSUM"))
        top = octx.enter_context(tc.tile_pool(name="tp_ps", bufs=2, space="PSUM"))

        # ---------------- constants ----------------
        invf_sb = consts.tile([P, 1], f32)
        nc.scalar.dma_start(out=invf_sb[:], in_=invf[:])
        pA_sb = consts.tile([P, P], f32r)
        nc.scalar.dma_start(out=pA_sb[:], in_=permA[:])
        pB_sb = consts.tile([P, P], f32r)
        nc.scalar.dma_start(out=pB_sb[:], in_=permB[:])
        ident_sb = consts.tile([P, P], bf16)
        nc.scalar.dma_start(out=ident_sb[:], in_=identT[:])
        woT_sb = consts.tile([P, ECH, D], bf16)
        nc.scalar.dma_start(out=woT_sb[:], in_=woT[:])
        pi2_sb = consts.tile([P, 1], f32)
        nc.vector.memset(pi2_sb[:], float(np.pi / 2))
        ones_sb = consts.tile([P, 1], f32r)
        nc.sync.dma_start(out=ones_sb[:], in_=onesc[:])
        onesb_sb = consts.tile([P, 1], bf16)
        nc.sync.dma_start(out=onesb_sb[:], in_=onesb[:])

        # persistent state
        xn_sb = persist.tile([P, DC, N], f32r)      # raw x, feature-major
        wq_sb = persist.tile([P, DC, E], f32r, tag="wq")
        nc.sync.dma_start(out=wq_sb[:], in_=wqT[:])
        wk_sb = persist.tile([P, DC, E], f32r, tag="wk")
        nc.gpsimd.dma_start(out=wk_sb[:], in_=wkT[:])
        wv_sb = persist.tile([P, DC, E], f32r, tag="wv")
        nc.gpsimd.dma_start(out=wv_sb[:], in_=wvT[:])
        kr_sb = persist.tile([P, ECH, N], bf16)     # rotated k, feature-major
        v_sb = persist.tile([P, KCH, HG * 65], bf16)  # token-major v | rowsum-ones
        costab = persist.tile([P, N], f32)          # becomes r*cos
        sintab = persist.tile([P, N], f32)          # becomes r*sin
        r_tok = persist.tile([P, KCH], f32)         # r, token-major

        sums_d = dram.tile([TCH, 2 * QW], f32)   # per t: [sum(512) | sumsq(512)]
        r_d = dram.tile([1, N], f32)

        # ones columns of v (rowsum denominators)
        for h in range(HG):
            nc.gpsimd.dma_start(
                out=v_sb[:, :, h * 65 + 64: h * 65 + 65],
                in_=onesb[:, 0:1][:, :, None].to_broadcast((P, KCH, 1)))

        # x loads (feature-major)
        for t, eng in zip(range(TCH), (nc.sync, nc.gpsimd, nc.sync, nc.gpsimd)):
            eng.dma_start(out=xn_sb[:, :, t * QW:(t + 1) * QW], in_=xT[t])

        # ---------------- RoPE trig tables (pure, no r yet) ----------------
        with ExitStack() as ptab:
            tblp = ptab.enter_context(tc.tile_pool(name="tbl", bufs=1))
            ftab = tblp.tile([P, N], f32)
            for blk in range(4):
                axis = blk % 2
                nc.sync.dma_start(
                    out=ftab[32 * blk: 32 * blk + 32, :],
                    in_=coordsT[axis: axis + 1, :].to_broadcast((32, N)),
                )
            nc.vector.tensor_scalar_mul(ftab[:], ftab[:], invf_sb[:])
            # round-to-nearest via +-2^23; costab doubles as the scratch
            nc.vector.tensor_scalar(
                costab[:], ftab[:], 1.0 / TWO_PI, MAGIC, AL.mult, AL.add)
            nc.vector.tensor_scalar_sub(costab[:], costab[:], MAGIC)
            nc.vector.scalar_tensor_tensor(
                sintab[:], costab[:], -TWO_PI, ftab[:], AL.mult, AL.add)
            nc.scalar.activation(sintab[:], sintab[:], AF.Sin)
            nc.vector.tensor_scalar(
                costab[:], ftab[:], 1.0 / TWO_PI, 0.25, AL.mult, AL.add)
            nc.vector.tensor_scalar_add(costab[:], costab[:], MAGIC)
            nc.vector.tensor_scalar_sub(costab[:], costab[:], MAGIC)
            nc.vector.scalar_tensor_tensor(
                costab[:], costab[:], -TWO_PI, ftab[:], AL.mult, AL.add)
            nc.scalar.activation(costab[:], costab[:], AF.Sin, bias=pi2_sb[:])

        # working pools, created after the table scratch is released
        rawp = octx.enter_context(tc.tile_pool(name="raw", bufs=2))
        cmbp = octx.enter_context(tc.tile_pool(name="cmb", bufs=2))
        ptp = octx.enter_context(tc.tile_pool(name="pt", bufs=2))
        qrp = octx.enter_context(tc.tile_pool(name="qr", bufs=2))
        attk = octx.enter_context(tc.tile_pool(name="attk", bufs=2))
        attf = octx.enter_context(tc.tile_pool(name="attf", bufs=2))
        ostg = octx.enter_context(tc.tile_pool(name="ostg", bufs=2))
        rrep = octx.enter_context(tc.tile_pool(name="rrep", bufs=2))
        sqp = octx.enter_context(tc.tile_pool(name="xsq", bufs=2))
        stcp = octx.enter_context(tc.tile_pool(name="stc", bufs=2))
        smallp = octx.enter_context(tc.tile_pool(name="small", bufs=1))
        rcps = octx.enter_context(tc.tile_pool(name="rcp", bufs=4))

        # ---------------- LN stats on raw x ----------------
        for t in range(TCH):
            tsl = slice(t * QW, (t + 1) * QW)
            st = scp.tile([P, 2 * QW], f32, space="PSUM", tag="sc")
            for dc in range(DC):
                xsq = sqp.tile([P, QW], bf16)
                nc.vector.tensor_mul(xsq[:], xn_sb[:, dc, tsl], xn_sb[:, dc, tsl])
                nc.tensor.matmul(
                    st[0:1, 0:QW], ones_sb[:], xn_sb[:, dc, tsl],
                    start=(dc == 0), stop=(dc == DC - 1))
                nc.tensor.matmul(
                    st[0:1, QW:2 * QW], onesb_sb[:], xsq[:],
                    start=(dc == 0), stop=(dc == DC - 1))
            stc = stcp.tile([1, 2 * QW], f32)
            nc.vector.tensor_copy(out=stc[:], in_=st[0:1, :])
            nc.sync.dma_start(out=sums_d[t:t + 1, :], in_=stc[:])

        # token-major r = rsqrt(var + eps)
        stT = smallp.tile([P, 2, KCH], f32)
        for s in range(2):
            for t in range(TCH):
                nc.sync.dma_start(
                    out=stT[:, s, t * 4:(t + 1) * 4],
                    in_=sums_d[t:t + 1, s * QW:(s + 1) * QW].rearrange(
                        "o (c p) -> p (o c)", p=P))
        mu = smallp.tile([P, KCH], f32)
        nc.vector.tensor_scalar_mul(mu[:], stT[:, 0, :], 1.0 / D)
        var = smallp.tile([P, KCH], f32)
        nc.vector.tensor_scalar(
            var[:], stT[:, 1, :], 1.0 / D, float(LN_EPS), AL.mult, AL.add)
        musq = smallp.tile([P, KCH], f32)
        nc.vector.tensor_mul(musq[:], mu[:], mu[:])
        nc.vector.tensor_sub(var[:], var[:], musq[:])
        sdev = smallp.tile([P, KCH], f32)
        nc.scalar.activation(sdev[:], var[:], AF.Sqrt)
        nc.vector.reciprocal(r_tok[:], sdev[:])
        nc.sync.dma_start(
            out=r_d.rearrange("o (c p) -> p (o c)", p=P), in_=r_tok[:])

        # fold r into the trig tables (feature-major broadcast)
        for t in range(TCH):
            tsl = slice(t * QW, (t + 1) * QW)
            rr = rrep.tile([P, QW], f32)
            nc.sync.dma_start(out=rr[:], in_=r_d[0:1, tsl].to_broadcast((P, QW)))
            nc.vector.tensor_mul(costab[:, tsl], costab[:, tsl], rr[:])
            nc.vector.tensor_mul(sintab[:, tsl], sintab[:, tsl], rr[:])

        # ---------------- q/k/v builders ----------------
        def qk_build_ops(w_sb, dst, ec, t):
            """Micro-op closures for one projected+rotated [128, QW] tile."""
            tsl = slice(t * QW, (t + 1) * QW)
            esl = slice(ec * P, (ec + 1) * P)
            state = {}
            ops = []

            def mk_proj(dc):
                def _op():
                    if dc == 0:
                        state["pj"] = top.tile([P, QW], f32, space="PSUM", tag="tp", name="pj")
                    nc.tensor.matmul(
                        state["pj"][:], w_sb[:, dc, esl], xn_sb[:, dc, tsl],
                        start=(dc == 0), stop=(dc == DC - 1))
                return _op
            for dc in range(DC):
                ops.append(mk_proj(dc))

            def op_raw():
                state["raw"] = rawp.tile([P, QW], f32r, name="raw")
                nc.vector.tensor_copy(out=state["raw"][:], in_=state["pj"][:])
            ops.append(op_raw)

            def op_ep():
                state["ep"] = top.tile([P, QW], f32, space="PSUM", tag="tp", name="ep")
                nc.tensor.matmul(state["ep"][:], pA_sb[:], state["raw"][:],
                                 start=True, stop=True)
            ops.append(op_ep)

            def op_t1():
                state["t1"] = cmbp.tile([P, QW], f32, tag="t1", name="t1")
                nc.vector.tensor_mul(state["t1"][:], state["ep"][:], costab[:, tsl])
            ops.append(op_t1)

            def op_op():
                state["op"] = top.tile([P, QW], f32, space="PSUM", tag="tp", name="opm")
                nc.tensor.matmul(state["op"][:], pB_sb[:], state["raw"][:],
                                 start=True, stop=True)
            ops.append(op_op)

            def op_t2():
                state["t2"] = cmbp.tile([P, QW], f32, tag="t2", name="t2")
                nc.vector.tensor_mul(state["t2"][:], state["op"][:], sintab[:, tsl])
            ops.append(op_t2)

            def op_add():
                nc.gpsimd.tensor_add(dst[:, ec, :] if dst.shape[2] == QW
                                     else dst[:, ec, tsl],
                                     state["t1"][:], state["t2"][:])
            ops.append(op_add)
            return ops

        def run_all(ops):
            for op in ops:
                op()

        # k: fully, before attention
        for ec in range(ECH):
            for t in range(TCH):
                run_all(qk_build_ops(wk_sb, kr_sb, ec, t))

        # v: token-major with r scale
        for kc in range(KCH):
            ksl = slice(kc * P, (kc + 1) * P)
            vp = scp.tile([P, 2 * QW], f32, space="PSUM", tag="sc")
            for dc in range(DC):
                nc.tensor.matmul(
                    vp[:, 0:E], xn_sb[:, dc, ksl], wv_sb[:, dc, :],
                    start=(dc == 0), stop=(dc == DC - 1))
            vdst = v_sb[:, kc, :].rearrange("p (h c) -> p h c", c=65)[:, :, 0:64]
            nc.vector.tensor_scalar_mul(
                vdst, vp[:, 0:E].rearrange("p (h c) -> p h c", c=DH),
                r_tok[:, kc:kc + 1])

        # q tiles per token chunk (t0 eagerly; t+1 woven into attention of t)
        qr_tiles = {}

        def make_q(t):
            qr_tiles[t] = qrp.tile([P, ECH, QW], bf16, name=f"qr{t}")
            ops = []
            for ec in range(ECH):
                ops.extend(qk_build_ops(wq_sb, qr_tiles[t], ec, t))
            return ops

        run_all(make_q(0))
        if KDBG:
            nc.sync.dma_start(out=dbg_r[:], in_=r_tok[:])
            nc.sync.dma_start(out=dbg_cos[:], in_=costab[:])
            nc.sync.dma_start(out=dbg_sin[:], in_=sintab[:])
            nc.sync.dma_start(out=dbg_kr[:], in_=kr_sb[:])
            nc.sync.dma_start(out=dbg_v[:], in_=v_sb[:])
            nc.sync.dma_start(out=dbg_q0[:], in_=qr_tiles[0][:])

        # ---------------- attention + out-projection ----------------
        for t in range(TCH):
            tsl = slice(t * QW, (t + 1) * QW)
            qr_t = qr_tiles[t]
            pend = make_q(t + 1) if t + 1 < TCH else []
            af_t = attf.tile([P, ECH, QW], bf16)        # feature-major attn out

            for pr in range(NPAIR):
                hA, hB = 2 * pr, 2 * pr + 1
                # zero once: a matmul start=True would zero the whole
                # bank, wiping sibling qc-regions' partial accumulations
                avA = avp.tile([P, 4 * 65], f32, space="PSUM", tag="av")
                avB = avp.tile([P, 4 * 65], f32, space="PSUM", tag="av")
                nc.vector.memset(avA[:], 0.0)
                nc.vector.memset(avB[:], 0.0)
                sc_t = [None, None]
                pt_t = [None, None]

                def scores(kcp):
                    sA = scp.tile([P, 2 * QW], f32, space="PSUM", tag="sc")
                    sB = scp.tile([P, 2 * QW], f32, space="PSUM", tag="sc")
                    for half in range(2):
                        kc = 2 * kcp + half
                        ksl = slice(kc * P, (kc + 1) * P)
                        hsl = slice(half * QW, (half + 1) * QW)
                        nc.tensor.matmul(
                            sA[:, hsl], kr_sb[0:64, pr, ksl], qr_t[0:64, pr, :],
                            start=True, stop=True, tile_position=(0, 0))
                        nc.tensor.matmul(
                            sB[:, hsl], kr_sb[64:128, pr, ksl], qr_t[64:128, pr, :],
                            start=True, stop=True, tile_position=(64, 0))
                    pA = ptp.tile([P, 2 * QW], bf16, name="ptA")
                    nc.scalar.activation(pA[:], sA[:], AF.Exp, scale=SCALE)
                    pB = ptp.tile([P, 2 * QW], bf16, name="ptB")
                    nc.scalar.activation(pB[:], sB[:], AF.Exp, scale=SCALE)
                    sc_t[0], sc_t[1] = sA, sB
                    pt_t[0], pt_t[1] = pA, pB

                def avmm(kcp):
                    for half in range(2):
                        kc = 2 * kcp + half
                        for pt_, av_, h in ((pt_t[0], avA, hA), (pt_t[1], avB, hB)):
                            for qc in range(4):
                                nc.tensor.matmul(
                                    av_[:, qc * 65:(qc + 1) * 65],
                                    pt_[:, half * QW + qc * P: half * QW + (qc + 1) * P],
                                    v_sb[:, kc, h * 65:(h + 1) * 65],
                                    start=False, stop=(kc == KCH - 1),
                                    skip_group_check=True)

                scores(0)
                for kcp in range(8):
                    prev_pt = (pt_t[0], pt_t[1])
                    if kcp < 7:
                        # issue next round's scores first so ACT never starves
                        scores_prev = (sc_t[0], sc_t[1])
                        scores(kcp + 1)
                    # AV of round kcp (uses prev_pt)
                    pt_save = (pt_t[0], pt_t[1])
                    pt_t[0], pt_t[1] = prev_pt
                    avmm(kcp)
                    pt_t[0], pt_t[1] = pt_save
                    # weave one pending q-build micro-op into the slack
                    if pend:
                        pend.pop(0)()

                # normalize token-major (bf16), then transpose this head
                # pair's 128 features back to feature-major
                at_p = attk.tile([P, TCH, 2 * DH], bf16)  # [qtok, qc, (2h d)]
                for hh, av_ in ((0, avA), (1, avB)):
                    for qc in range(4):
                        rc = rcps.tile([P, 1], f32)
                        nc.vector.reciprocal(rc[:], av_[:, qc * 65 + 64: qc * 65 + 65])
                        nc.vector.tensor_scalar_mul(
                            at_p[:, qc, hh * DH:(hh + 1) * DH],
                            av_[:, qc * 65: qc * 65 + 64], rc[:])
                tp = top.tile([P, QW], bf16, space="PSUM", tag="tp")
                for qc in range(4):
                    nc.tensor.transpose(
                        tp[:, qc * P:(qc + 1) * P], at_p[:, qc, :], ident_sb[:])
                nc.vector.tensor_copy(out=af_t[:, pr, :], in_=tp[:])

            run_all(pend)
            pend = []
            if KDBG and t == 0:
                nc.sync.dma_start(out=dbg_at[:], in_=af_t[:])

            # partial out-projection
            for dmc in range(DC):
                op_ = top.tile([P, QW], f32, space="PSUM", tag="tp")
                for ec in range(ECH):
                    nc.tensor.matmul(
                        op_[:], woT_sb[:, ec, dmc * P:(dmc + 1) * P],
                        af_t[:, ec, :], start=(ec == 0), stop=(ec == ECH - 1))
                og = ostg.tile([P, QW], f32)
                nc.vector.tensor_copy(out=og[:], in_=op_[:])
                nc.sync.dma_start(out=outT_r[:, dmc, tsl], in_=og[:])

    nc.compile()
    return nc


def _host_constants():
    j = np.arange(P) % 16
    invf = (ROPE_BASE ** (-(j / 16.0))).astype(np.float32).reshape(P, 1)
    A = np.zeros((P, P), np.float32)
    Bm = np.zeros((P, P), np.float32)
    for p in range(P):
        base = (p // 32) * 32
        jj = p % 32
        if jj < 16:
            A[p, base + 2 * jj] = 1.0
            Bm[p, base + 2 * jj + 1] = -1.0
        else:
            A[p, base + 2 * (jj - 16) + 1] = 1.0
            Bm[p, base + 2 * (jj - 16)] = 1.0
    return invf, np.ascontiguousarray(A.T), np.ascontiguousarray(Bm.T)


def _run(x, coords, ln_gamma, ln_beta, w_qkv, w_out, **run_kwargs):
    from concourse.bass_utils import run_bass_kernel_spmd

    x = np.asarray(x, np.float32)
    coords = np.asarray(coords, np.float32)
    ln_gamma = np.asarray(ln_gamma, np.float32)
    ln_beta = np.asarray(ln_beta, np.float32)
    w_qkv = np.asarray(w_qkv, np.float32)
    w_out = np.asarray(w_out, np.float32)
    assert not np.any(ln_beta != 0.0), "kernel assumes ln_beta == 0"

    if "g" not in _GRAPH_CACHE:
        _GRAPH_CACHE["g"] = _build_graph()
    nc = _GRAPH_CACHE["g"]

    invf, AT, BT = _host_constants()
    # fold ln_gamma, then center rows: (W - rowmean(W)) @ x == W @ (x - mu)
    wg = (w_qkv * ln_gamma[None, :]).astype(np.float32)
    wg = wg - wg.mean(axis=1, keepdims=True)
    wq, wk, wv = wg[0:D], wg[D:2 * D], wg[2 * D:3 * D]
    ident = np.eye(P, dtype=ml_dtypes.bfloat16)

    in_maps = []
    for core in range(8):
        b, g = core // 2, core % 2
        sl = slice(g * E, (g + 1) * E)
        m = {
            "xT": np.ascontiguousarray(
                x[b].T.reshape(DC, P, TCH, QW).transpose(2, 1, 0, 3)),
            "wqT": np.ascontiguousarray(
                wq[sl].T.reshape(DC, P, E).transpose(1, 0, 2)),
            "wkT": np.ascontiguousarray(
                wk[sl].T.reshape(DC, P, E).transpose(1, 0, 2)),
            "wvT": np.ascontiguousarray(
                wv[sl].T.reshape(DC, P, E).transpose(1, 0, 2)),
            "woT": np.ascontiguousarray(
                w_out[:, sl].T.reshape(ECH, P, D).transpose(1, 0, 2)
            ).astype(ml_dtypes.bfloat16),
            "coordsT": np.ascontiguousarray(coords[b].T),
            "invf": invf,
            "permA": AT,
            "permB": BT,
            "identT": ident,
            "onesc": np.ones((P, 1), np.float32),
            "onesb": np.ones((P, 1), dtype=ml_dtypes.bfloat16),
        }
        in_maps.append(m)

    res = run_bass_kernel_spmd(nc, in_maps, core_ids=list(range(8)), **run_kwargs)
    out = np.empty((B, N, D), np.float32)
    for b in range(B):
        acc = res.results[2 * b]["out"] + res.results[2 * b + 1]["out"]
        out[b] = acc.T
    return out, res


def kernel(x, coords, ln_gamma, ln_beta, w_qkv, w_out):
    out, _ = _run(x, coords, ln_gamma, ln_beta, w_qkv, w_out)
    return out
